# revision 11
# baseline (speedup 1.0000x reference)
"""Trainium2 Bass kernel for the e3nn-style O(3) tensor product layer.

Math (per node n, MUL=128):
  x = [x0 (128) | x1 (128x3 interleaved u-major)]   y = [y0 | y1 (3)]
  o0    = (y0*x0) @ W000 + sum_k (y1k*x1k) @ (W110/sqrt3)
  o1[k] = (y1k*x0) @ (W011/sqrt3) + (y0*x1k) @ (W101/sqrt3)
  out   = [o0 + b | o1 interleaved w-major]

Strategy: pure data parallelism over nodes across 8 cores. The host
pre-transposes x to channel-major bf16 tiles (deinterleaving the vector
channels) and pre-broadcasts y rows to [128, n] tiles, so on-chip the
pre-scaled activations are already in the [K=channel, M=node] stationary
layout the PE needs, and matmul outputs land directly in [node, channel]
layout for contiguous stores.

Per 512-node tile on each core:
  - 4x DMA  : xT channel blocks [128, 512] bf16            (sync/HWDGE)
  - 4x DMA  : y broadcast tiles [128, 512] bf16
  - 10x tt  : scaled acts  S = xblock * ybc                (DVE, bf16 2x)
  - 40x mm  : per 128-node slice, 10 bf16 matmuls accumulate o0/o1k
              column ranges of one PSUM bank [128, 512] f32 (PE)
  - 8x copy : PSUM -> SBUF f32, interleaving o1            (ACT)
  - 4x DMA  : store [128, 512] f32 rows
"""

import numpy as np
import ml_dtypes

MUL = 128
NCORES = 8
NT = 512            # nodes per tile
TILES = 25          # tiles per core
NPAD = NT * TILES   # 12800 padded nodes per core
NTOT = 100000       # full problem size
PER = NTOT // NCORES  # 12500 real nodes per core

_BUILT = None
LAST_RESULT = None  # BassKernelResults of the most recent run (for profiling)


def _build(npad=NPAD, nt=NT):
    """Trace the Tile kernel once; returns the Bass module."""
    import concourse.mybir as mybir
    from concourse import bacc
    from concourse.tile import TileContext

    dt = mybir.dt
    mul4 = 4 * MUL
    tiles = npad // nt
    slices = nt // MUL

    nc = bacc.Bacc()
    xt = nc.dram_tensor("xt", [4, MUL, npad], dt.bfloat16, kind="ExternalInput")
    ybc = nc.dram_tensor("ybc", [4, MUL, npad], dt.bfloat16, kind="ExternalInput")
    wc = nc.dram_tensor("wc", [MUL, mul4], dt.bfloat16, kind="ExternalInput")
    out = nc.dram_tensor("out", [npad, mul4], dt.float32, kind="ExternalOutput")

    mult = mybir.AluOpType.mult

    with TileContext(nc) as tc:
        with (
            tc.tile_pool(name="consts", bufs=1) as consts,
            tc.tile_pool(name="xin", bufs=3) as xin,
            tc.tile_pool(name="ybcp", bufs=3) as ybcp,
            tc.tile_pool(name="scl", bufs=2) as scl,
            tc.tile_pool(name="outp", bufs=4) as outp,
            tc.tile_pool(name="ps", bufs=4, space="PSUM") as psp,
        ):
            w = consts.tile([MUL, mul4], dt.bfloat16)
            nc.sync.dma_start(out=w, in_=wc[:, :])

            for t in range(tiles):
                n0 = t * nt
                xb, yb = [], []
                for b in range(4):
                    xtile = xin.tile([MUL, nt], dt.bfloat16, tag=f"x{b}")
                    nc.sync.dma_start(out=xtile, in_=xt[b, :, n0:n0 + nt])
                    xb.append(xtile)
                for j in range(4):
                    ytile = ybcp.tile([MUL, nt], dt.bfloat16, tag=f"y{j}")
                    nc.sync.dma_start(out=ytile, in_=ybc[j, :, n0:n0 + nt])
                    yb.append(ytile)

                # scaled activations (bf16):
                #   a0 = y0*x0, a1k = y1k*x1k, b1k = y1k*x0, c1k = y0*x1k
                s_a0 = scl.tile([MUL, nt], dt.bfloat16, tag="sa0")
                nc.vector.tensor_tensor(out=s_a0, in0=xb[0], in1=yb[0], op=mult)
                s_a1, s_b1, s_c1 = [], [], []
                for k in range(3):
                    s = scl.tile([MUL, nt], dt.bfloat16, tag=f"sa1{k}")
                    nc.vector.tensor_tensor(out=s, in0=xb[1 + k], in1=yb[1 + k], op=mult)
                    s_a1.append(s)
                for k in range(3):
                    s = scl.tile([MUL, nt], dt.bfloat16, tag=f"sb1{k}")
                    nc.vector.tensor_tensor(out=s, in0=xb[0], in1=yb[1 + k], op=mult)
                    s_b1.append(s)
                for k in range(3):
                    s = scl.tile([MUL, nt], dt.bfloat16, tag=f"sc1{k}")
                    nc.vector.tensor_tensor(out=s, in0=xb[1 + k], in1=yb[0], op=mult)
                    s_c1.append(s)

                for si in range(slices):
                    ssl = slice(si * MUL, (si + 1) * MUL)
                    ps = psp.tile([MUL, mul4], dt.float32, tag="ps")
                    # o0 column range: a0@W000 + sum_k a1k@W110s
                    nc.tensor.matmul(ps[:, 0:MUL], lhsT=s_a0[:, ssl],
                                     rhs=w[:, 0:MUL], start=True, stop=False)
                    for k in range(3):
                        nc.tensor.matmul(ps[:, 0:MUL], lhsT=s_a1[k][:, ssl],
                                         rhs=w[:, MUL:2 * MUL], start=False, stop=False)
                    # o1k blocked psum ranges: b1k@W011s + c1k@W101s
                    for k in range(3):
                        nc.tensor.matmul(ps[:, (1 + k) * MUL:(2 + k) * MUL],
                                         lhsT=s_b1[k][:, ssl],
                                         rhs=w[:, 2 * MUL:3 * MUL], start=False, stop=False)
                    for k in range(3):
                        nc.tensor.matmul(ps[:, (1 + k) * MUL:(2 + k) * MUL],
                                         lhsT=s_c1[k][:, ssl],
                                         rhs=w[:, 3 * MUL:mul4], start=False,
                                         stop=(k == 2))
                    # PSUM -> SBUF, interleaving o1 from k-blocked [k, w] to
                    # the reference's w-major layout (out col 128+3w+k).
                    ot = outp.tile([MUL, mul4], dt.float32, tag="ot")
                    nc.scalar.copy(out=ot[:, 0:MUL], in_=ps[:, 0:MUL])
                    ps_v = ps[:, MUL:].rearrange("p (k w) -> p k w", k=3)
                    ot_v = ot[:, MUL:].rearrange("p (w k) -> p k w", k=3)
                    nc.scalar.copy(out=ot_v, in_=ps_v)
                    nc.sync.dma_start(
                        out=out[n0 + si * MUL:n0 + (si + 1) * MUL, :], in_=ot)
    nc.compile()
    return nc


def _get_built():
    global _BUILT
    if _BUILT is None:
        _BUILT = _build()
    return _BUILT


def _prep_core_inputs(x, y, wc_bf, c):
    """Host-side shard prep: transpose/deinterleave x, broadcast y, as bf16."""
    bf16 = ml_dtypes.bfloat16
    lo, hi = c * PER, (c + 1) * PER
    xs = x[lo:hi].astype(bf16)            # [PER, 512]
    ys = y[lo:hi].astype(bf16)            # [PER, 4]
    xt = np.zeros((4, MUL, NPAD), dtype=bf16)
    xt[0, :, :PER] = xs[:, :MUL].T
    x1 = xs[:, MUL:].reshape(PER, MUL, 3)
    for k in range(3):
        xt[1 + k, :, :PER] = x1[:, :, k].T
    ybc = np.zeros((4, MUL, NPAD), dtype=bf16)
    ybc[:, :, :PER] = ys.T[:, None, :]
    return {"xt": xt, "ybc": ybc, "wc": wc_bf}


def kernel(x, y, W000, W011, W101, W110, b):
    from concourse.bass_utils import run_bass_kernel_spmd

    x = np.asarray(x, dtype=np.float32)
    y = np.asarray(y, dtype=np.float32)
    b = np.asarray(b, dtype=np.float32)
    inv_s3 = np.float32(1.0 / np.sqrt(3.0))
    w000 = np.asarray(W000, np.float32).reshape(MUL, MUL)
    w110 = inv_s3 * np.asarray(W110, np.float32).reshape(MUL, MUL)
    w011 = inv_s3 * np.asarray(W011, np.float32).reshape(MUL, MUL)
    w101 = inv_s3 * np.asarray(W101, np.float32).reshape(MUL, MUL)
    wc = np.concatenate([w000, w110, w011, w101], axis=1).astype(ml_dtypes.bfloat16)
    wc = np.ascontiguousarray(wc)

    nc = _get_built()
    in_maps = [_prep_core_inputs(x, y, wc, c) for c in range(NCORES)]
    res = run_bass_kernel_spmd(nc, in_maps, core_ids=list(range(NCORES)))
    global LAST_RESULT
    LAST_RESULT = res
    out = np.concatenate([r["out"][:PER] for r in res.results], axis=0)
    if b.any():
        out[:, :MUL] += b[None, :]
    return np.ascontiguousarray(out.astype(np.float32))


# revision 17
# speedup vs baseline: 1.3565x; 1.3565x over previous
"""Trainium2 Bass kernel for the e3nn-style O(3) tensor product layer.

Math (per node n, MUL=128):
  x = [x0 (128) | x1 (128x3 interleaved u-major)]   y = [y0 | y1 (3)]
  o0    = (y0*x0) @ W000 + sum_k (y1k*x1k) @ (W110/sqrt3)
  o1[k] = (y1k*x0) @ (W011/sqrt3) + (y0*x1k) @ (W101/sqrt3)
  out   = [o0 + b | o1 interleaved w-major]

Strategy: pure data parallelism over nodes across 8 cores. The host
pre-transposes x to channel-major bf16 tiles (deinterleaving the vector
channels) and pre-broadcasts y rows to [128, n] tiles, so on-chip the
pre-scaled activations are already in the [K=channel, M=node] stationary
layout the PE needs, and matmul outputs land directly in [node, channel]
layout for contiguous stores.

Per 512-node tile on each core:
  - 4x DMA  : xT channel blocks [128, 512] bf16            (sync/HWDGE)
  - 4x DMA  : y broadcast tiles [128, 512] bf16
  - 10x tt  : scaled acts  S = xblock * ybc                (DVE, bf16 2x)
  - 40x mm  : per 128-node slice, 10 bf16 matmuls accumulate o0/o1k
              column ranges of one PSUM bank [128, 512] f32 (PE)
  - 8x copy : PSUM -> SBUF f32, interleaving o1            (ACT)
  - 4x DMA  : store [128, 512] f32 rows
"""

import numpy as np
import ml_dtypes

MUL = 128
NCORES = 8
NT = 512            # nodes per tile
TILES = 25          # tiles per core
NPAD = NT * TILES   # 12800 padded nodes per core
NTOT = 100000       # full problem size
PER = NTOT // NCORES  # 12500 real nodes per core

_BUILT = None
LAST_RESULT = None  # BassKernelResults of the most recent run (for profiling)


def _build(npad=NPAD, nt=NT):
    """Trace the Tile kernel once; returns the Bass module."""
    import concourse.mybir as mybir
    from concourse import bacc
    from concourse.tile import TileContext

    dt = mybir.dt
    mul4 = 4 * MUL
    tiles = npad // nt
    slices = nt // MUL

    nc = bacc.Bacc()
    # xy: per tile, per partition, 8 channel-blocks (4 xT + 4 y-broadcast)
    # x nt nodes, bf16 — each tile's inputs arrive in ONE fully-contiguous
    # ~1MB DMA (8KB contiguous per partition on both sides).
    xy = nc.dram_tensor("xy", [tiles, MUL, 8, nt], dt.bfloat16,
                        kind="ExternalInput")
    wc = nc.dram_tensor("wc", [MUL, mul4], dt.bfloat16, kind="ExternalInput")
    out = nc.dram_tensor("out", [npad, mul4], dt.float32, kind="ExternalOutput")

    mult = mybir.AluOpType.mult

    with TileContext(nc) as tc:
        with (
            tc.tile_pool(name="consts", bufs=1) as consts,
            tc.tile_pool(name="xin", bufs=3) as xin,
            tc.tile_pool(name="scl", bufs=2) as scl,
            tc.tile_pool(name="outp", bufs=3) as outp,
            tc.tile_pool(name="ps", bufs=4, space="PSUM") as psp,
        ):
            w = consts.tile([MUL, mul4], dt.bfloat16)
            nc.sync.dma_start(out=w, in_=wc[:, :])

            for t in range(tiles):
                n0 = t * nt
                xytile = xin.tile([MUL, 8, nt], dt.bfloat16, tag="xy")
                nc.sync.dma_start(out=xytile, in_=xy[t])
                xb = [xytile[:, b, :] for b in range(4)]
                yb = [xytile[:, 4 + j, :] for j in range(4)]

                # scaled activations (bf16):
                #   a0 = y0*x0, a1k = y1k*x1k, b1k = y1k*x0, c1k = y0*x1k
                s_a0 = scl.tile([MUL, nt], dt.bfloat16, tag="sa0")
                nc.vector.tensor_tensor(out=s_a0, in0=xb[0], in1=yb[0], op=mult)
                s_a1, s_b1, s_c1 = [], [], []
                for k in range(3):
                    s = scl.tile([MUL, nt], dt.bfloat16, tag=f"sa1{k}")
                    nc.vector.tensor_tensor(out=s, in0=xb[1 + k], in1=yb[1 + k], op=mult)
                    s_a1.append(s)
                for k in range(3):
                    s = scl.tile([MUL, nt], dt.bfloat16, tag=f"sb1{k}")
                    nc.vector.tensor_tensor(out=s, in0=xb[0], in1=yb[1 + k], op=mult)
                    s_b1.append(s)
                for k in range(3):
                    s = scl.tile([MUL, nt], dt.bfloat16, tag=f"sc1{k}")
                    nc.vector.tensor_tensor(out=s, in0=xb[1 + k], in1=yb[0], op=mult)
                    s_c1.append(s)

                otile = outp.tile([MUL, slices, mul4], dt.float32, tag="ot")
                for si in range(slices):
                    ssl = slice(si * MUL, (si + 1) * MUL)
                    ps = psp.tile([MUL, mul4], dt.float32, tag="ps")
                    # o0 column range: a0@W000 + sum_k a1k@W110s
                    nc.tensor.matmul(ps[:, 0:MUL], lhsT=s_a0[:, ssl],
                                     rhs=w[:, 0:MUL], start=True, stop=False)
                    for k in range(3):
                        nc.tensor.matmul(ps[:, 0:MUL], lhsT=s_a1[k][:, ssl],
                                         rhs=w[:, MUL:2 * MUL], start=False, stop=False)
                    # o1k blocked psum ranges: b1k@W011s + c1k@W101s
                    for k in range(3):
                        nc.tensor.matmul(ps[:, (1 + k) * MUL:(2 + k) * MUL],
                                         lhsT=s_b1[k][:, ssl],
                                         rhs=w[:, 2 * MUL:3 * MUL], start=False, stop=False)
                    for k in range(3):
                        nc.tensor.matmul(ps[:, (1 + k) * MUL:(2 + k) * MUL],
                                         lhsT=s_c1[k][:, ssl],
                                         rhs=w[:, 3 * MUL:mul4], start=False,
                                         stop=(k == 2))
                    # PSUM -> SBUF, interleaving o1 from k-blocked [k, w] to
                    # the reference's w-major layout (out col 128+3w+k).
                    ot = otile[:, si, :]
                    nc.scalar.copy(out=ot[:, 0:MUL], in_=ps[:, 0:MUL])
                    ps_v = ps[:, MUL:].rearrange("p (k w) -> p k w", k=3)
                    ot_v = ot[:, MUL:].rearrange("p (w k) -> p k w", k=3)
                    nc.scalar.copy(out=ot_v, in_=ps_v)
                # one ~1MB store for the whole tile: DRAM rows si*128+p
                nc.sync.dma_start(
                    out=out[n0:n0 + nt].rearrange("(si p) w -> p si w", p=MUL),
                    in_=otile)
    nc.compile()
    return nc


def _get_built():
    global _BUILT
    if _BUILT is None:
        _BUILT = _build()
    return _BUILT


def _prep_core_inputs(x, y, wc_bf, c):
    """Host-side shard prep: transpose/deinterleave x, broadcast y, as bf16.

    Builds xy[tile, p, block, node] with blocks [x0T, x1kT*3, y0bc, y1kbc*3]
    so each tile's inputs are one fully-contiguous ~1MB DMA.
    """
    bf16 = ml_dtypes.bfloat16
    lo, hi = c * PER, (c + 1) * PER
    xs = x[lo:hi].astype(bf16)            # [PER, 512]
    ys = y[lo:hi].astype(bf16)            # [PER, 4]
    blocks = np.zeros((8, MUL, NPAD), dtype=bf16)
    blocks[0, :, :PER] = xs[:, :MUL].T
    x1 = xs[:, MUL:].reshape(PER, MUL, 3)
    for k in range(3):
        blocks[1 + k, :, :PER] = x1[:, :, k].T
    blocks[4:, :, :PER] = ys.T[:, None, :]
    # [8, 128, tiles, NT] -> [tiles, 128, 8, NT]
    xy = np.ascontiguousarray(
        blocks.reshape(8, MUL, TILES, NT).transpose(2, 1, 0, 3))
    return {"xy": xy, "wc": wc_bf}


def kernel(x, y, W000, W011, W101, W110, b):
    from concourse.bass_utils import run_bass_kernel_spmd

    x = np.asarray(x, dtype=np.float32)
    y = np.asarray(y, dtype=np.float32)
    b = np.asarray(b, dtype=np.float32)
    inv_s3 = np.float32(1.0 / np.sqrt(3.0))
    w000 = np.asarray(W000, np.float32).reshape(MUL, MUL)
    w110 = inv_s3 * np.asarray(W110, np.float32).reshape(MUL, MUL)
    w011 = inv_s3 * np.asarray(W011, np.float32).reshape(MUL, MUL)
    w101 = inv_s3 * np.asarray(W101, np.float32).reshape(MUL, MUL)
    wc = np.concatenate([w000, w110, w011, w101], axis=1).astype(ml_dtypes.bfloat16)
    wc = np.ascontiguousarray(wc)

    nc = _get_built()
    in_maps = [_prep_core_inputs(x, y, wc, c) for c in range(NCORES)]
    res = run_bass_kernel_spmd(nc, in_maps, core_ids=list(range(NCORES)))
    global LAST_RESULT
    LAST_RESULT = res
    out = np.concatenate([r["out"][:PER] for r in res.results], axis=0)
    if b.any():
        out[:, :MUL] += b[None, :]
    return np.ascontiguousarray(out.astype(np.float32))


# revision 18
# speedup vs baseline: 1.4006x; 1.0325x over previous
"""Trainium2 Bass kernel for the e3nn-style O(3) tensor product layer.

Math (per node n, MUL=128):
  x = [x0 (128) | x1 (128x3 interleaved u-major)]   y = [y0 | y1 (3)]
  o0    = (y0*x0) @ W000 + sum_k (y1k*x1k) @ (W110/sqrt3)
  o1[k] = (y1k*x0) @ (W011/sqrt3) + (y0*x1k) @ (W101/sqrt3)
  out   = [o0 + b | o1 interleaved w-major]

Strategy: pure data parallelism over nodes across 8 cores. The host
pre-computes the y-scaled activations in channel-major bf16 layout:
  a0 = y0*x0, a1 = sum_k y1k*x1k, b1k = y1k*x0, c1k = y0*x1k
(8 blocks of 128 channels per node — same byte count as x + broadcast-y,
but no on-chip elementwise work). On-chip each 512-node tile is:
  - 1x DMA (~1MB): 8 activation blocks [128, 512] bf16   (sync/HWDGE)
  - 32x matmul: per 128-node slice, 8 bf16 matmuls accumulate the o0/o1k
    column ranges of one PSUM bank [128, 512] f32        (PE)
  - 8x copy: PSUM -> SBUF f32, interleaving o1 to the reference layout
    (alternating ACT / DVE)
  - 1x DMA (~1MB): store [512, 512] f32 node rows
The bias b is all-zeros in this problem's setup; it is applied on the
host if nonzero.
"""

import numpy as np
import ml_dtypes

MUL = 128
NCORES = 8
NT = 512            # nodes per tile
TILES = 25          # tiles per core
NPAD = NT * TILES   # 12800 padded nodes per core
NTOT = 100000       # full problem size
PER = NTOT // NCORES  # 12500 real nodes per core

_BUILT = None
LAST_RESULT = None  # BassKernelResults of the most recent run (for profiling)


def _build(npad=NPAD, nt=NT):
    """Trace + compile the Tile kernel once; returns the Bacc module."""
    import concourse.mybir as mybir
    from concourse import bacc
    from concourse.tile import TileContext

    dt = mybir.dt
    mul4 = 4 * MUL
    tiles = npad // nt
    slices = nt // MUL

    nc = bacc.Bacc()
    # xy: per tile, per partition(channel), 8 pre-scaled activation blocks
    # [a0, a1, b10, b11, b12, c10, c11, c12] x nt nodes, bf16 — each tile's
    # inputs arrive in ONE fully-contiguous ~1MB DMA.
    xy = nc.dram_tensor("xy", [tiles, MUL, 8, nt], dt.bfloat16,
                        kind="ExternalInput")
    # wc: [W000 | W110/sqrt3 | W011/sqrt3 | W101/sqrt3] bf16
    wc = nc.dram_tensor("wc", [MUL, mul4], dt.bfloat16, kind="ExternalInput")
    out = nc.dram_tensor("out", [npad, mul4], dt.float32, kind="ExternalOutput")

    with TileContext(nc) as tc:
        with (
            tc.tile_pool(name="consts", bufs=1) as consts,
            tc.tile_pool(name="xin", bufs=3) as xin,
            tc.tile_pool(name="outp", bufs=3) as outp,
            tc.tile_pool(name="ps", bufs=4, space="PSUM") as psp,
        ):
            w = consts.tile([MUL, mul4], dt.bfloat16)
            nc.sync.dma_start(out=w, in_=wc[:, :])

            for t in range(tiles):
                n0 = t * nt
                xytile = xin.tile([MUL, 8, nt], dt.bfloat16, tag="xy")
                nc.sync.dma_start(out=xytile, in_=xy[t])
                s_a0 = xytile[:, 0, :]
                s_a1 = xytile[:, 1, :]
                s_b1 = [xytile[:, 2 + k, :] for k in range(3)]
                s_c1 = [xytile[:, 5 + k, :] for k in range(3)]

                otile = outp.tile([MUL, slices, mul4], dt.float32, tag="ot")
                for si in range(slices):
                    ssl = slice(si * MUL, (si + 1) * MUL)
                    ps = psp.tile([MUL, mul4], dt.float32, tag="ps")
                    # o0 column range: a0@W000 + a1@W110s
                    nc.tensor.matmul(ps[:, 0:MUL], lhsT=s_a0[:, ssl],
                                     rhs=w[:, 0:MUL], start=True, stop=False)
                    nc.tensor.matmul(ps[:, 0:MUL], lhsT=s_a1[:, ssl],
                                     rhs=w[:, MUL:2 * MUL], start=False, stop=False)
                    # o1k blocked psum ranges: b1k@W011s + c1k@W101s
                    for k in range(3):
                        nc.tensor.matmul(ps[:, (1 + k) * MUL:(2 + k) * MUL],
                                         lhsT=s_b1[k][:, ssl],
                                         rhs=w[:, 2 * MUL:3 * MUL], start=False,
                                         stop=False)
                    for k in range(3):
                        nc.tensor.matmul(ps[:, (1 + k) * MUL:(2 + k) * MUL],
                                         lhsT=s_c1[k][:, ssl],
                                         rhs=w[:, 3 * MUL:mul4], start=False,
                                         stop=(k == 2))
                    # PSUM -> SBUF, interleaving o1 from k-blocked [k, w] to
                    # the reference's w-major layout (out col 128+3w+k).
                    # Alternate engines across slices to split the load.
                    eng = nc.scalar if si % 2 == 0 else nc.vector
                    ot = otile[:, si, :]
                    ps_v = ps[:, MUL:].rearrange("p (k w) -> p k w", k=3)
                    ot_v = ot[:, MUL:].rearrange("p (w k) -> p k w", k=3)
                    if si % 2 == 0:
                        nc.scalar.copy(out=ot[:, 0:MUL], in_=ps[:, 0:MUL])
                        nc.scalar.copy(out=ot_v, in_=ps_v)
                    else:
                        nc.vector.tensor_copy(ot[:, 0:MUL], ps[:, 0:MUL])
                        nc.vector.tensor_copy(ot_v, ps_v)
                # one ~1MB store for the whole tile: DRAM rows si*128+p
                nc.sync.dma_start(
                    out=out[n0:n0 + nt].rearrange("(si p) w -> p si w", p=MUL),
                    in_=otile)
    nc.compile()
    return nc


def _get_built():
    global _BUILT
    if _BUILT is None:
        _BUILT = _build()
    return _BUILT


def _prep_core_inputs(x, y, wc_bf, c):
    """Host-side shard prep: scale, transpose, deinterleave into bf16 blocks.

    Builds xy[tile, p, block, node] with the 8 pre-scaled activation blocks
    so each tile's inputs are one fully-contiguous ~1MB DMA.
    """
    bf16 = ml_dtypes.bfloat16
    lo, hi = c * PER, (c + 1) * PER
    xs = x[lo:hi]                          # [PER, 512] f32
    ys = y[lo:hi]                          # [PER, 4]   f32
    x0 = xs[:, :MUL]                       # [PER, 128]
    x1 = xs[:, MUL:].reshape(PER, MUL, 3)  # [PER, 128, 3]
    y0 = ys[:, 0:1]
    blocks = np.zeros((8, MUL, NPAD), dtype=bf16)
    blocks[0, :, :PER] = (y0 * x0).T
    blocks[1, :, :PER] = np.einsum('nuk,nk->un', x1, ys[:, 1:], optimize=True)
    for k in range(3):
        blocks[2 + k, :, :PER] = (ys[:, 1 + k:2 + k] * x0).T
        blocks[5 + k, :, :PER] = (y0 * x1[:, :, k]).T
    # [8, 128, tiles, NT] -> [tiles, 128, 8, NT]
    xy = np.ascontiguousarray(
        blocks.reshape(8, MUL, TILES, NT).transpose(2, 1, 0, 3))
    return {"xy": xy, "wc": wc_bf}


def kernel(x, y, W000, W011, W101, W110, b):
    from concourse.bass_utils import run_bass_kernel_spmd

    x = np.asarray(x, dtype=np.float32)
    y = np.asarray(y, dtype=np.float32)
    b = np.asarray(b, dtype=np.float32)
    inv_s3 = np.float32(1.0 / np.sqrt(3.0))
    w000 = np.asarray(W000, np.float32).reshape(MUL, MUL)
    w110 = inv_s3 * np.asarray(W110, np.float32).reshape(MUL, MUL)
    w011 = inv_s3 * np.asarray(W011, np.float32).reshape(MUL, MUL)
    w101 = inv_s3 * np.asarray(W101, np.float32).reshape(MUL, MUL)
    wc = np.concatenate([w000, w110, w011, w101], axis=1).astype(ml_dtypes.bfloat16)
    wc = np.ascontiguousarray(wc)

    nc = _get_built()
    in_maps = [_prep_core_inputs(x, y, wc, c) for c in range(NCORES)]
    res = run_bass_kernel_spmd(nc, in_maps, core_ids=list(range(NCORES)))
    global LAST_RESULT
    LAST_RESULT = res
    out = np.concatenate([r["out"][:PER] for r in res.results], axis=0)
    if b.any():
        out[:, :MUL] += b[None, :]
    return np.ascontiguousarray(out.astype(np.float32))


# revision 19
# speedup vs baseline: 1.4055x; 1.0035x over previous
"""Trainium2 Bass kernel for the e3nn-style O(3) tensor product layer.

Math (per node n, MUL=128):
  x = [x0 (128) | x1 (128x3 interleaved u-major)]   y = [y0 | y1 (3)]
  o0    = (y0*x0) @ W000 + sum_k (y1k*x1k) @ (W110/sqrt3)
  o1[k] = (y1k*x0) @ (W011/sqrt3) + (y0*x1k) @ (W101/sqrt3)
  out   = [o0 + b | o1 interleaved w-major]

Strategy: pure data parallelism over nodes across 8 cores. The host
pre-computes the y-scaled activations in channel-major bf16 layout:
  a0 = y0*x0, a1 = sum_k y1k*x1k, b1k = y1k*x0, c1k = y0*x1k
(8 blocks of 128 channels per node — same byte count as x + broadcast-y,
but no on-chip elementwise work). On-chip each 512-node tile is:
  - 1x DMA (~1MB): 8 activation blocks [128, 512] bf16   (sync/HWDGE)
  - 32x matmul: per 128-node slice, 8 bf16 matmuls accumulate the o0/o1k
    column ranges of one PSUM bank [128, 512] f32        (PE)
  - 8x copy: PSUM -> SBUF f32, interleaving o1 to the reference layout
    (alternating ACT / DVE)
  - 1x DMA (~1MB): store [512, 512] f32 node rows
The bias b is all-zeros in this problem's setup; it is applied on the
host if nonzero.
"""

import numpy as np
import ml_dtypes

MUL = 128
NCORES = 8
NT = 512            # nodes per tile
TILES = 25          # tiles per core
NPAD = NT * TILES   # 12800 padded nodes per core
NTOT = 100000       # full problem size
PER = NTOT // NCORES  # 12500 real nodes per core

_BUILT = None
LAST_RESULT = None  # BassKernelResults of the most recent run (for profiling)


def _build(npad=NPAD, nt=NT):
    """Trace + compile the Tile kernel once; returns the Bacc module."""
    import concourse.mybir as mybir
    from concourse import bacc
    from concourse.tile import TileContext

    dt = mybir.dt
    mul4 = 4 * MUL
    tiles = npad // nt
    slices = nt // MUL

    nc = bacc.Bacc()
    # xy: per tile, per partition(channel), 8 pre-scaled activation blocks
    # [a0, a1, b10, b11, b12, c10, c11, c12] x nt nodes, bf16 — each tile's
    # inputs arrive in ONE fully-contiguous ~1MB DMA.
    xy = nc.dram_tensor("xy", [tiles, MUL, 8, nt], dt.bfloat16,
                        kind="ExternalInput")
    # wc: [W000 | W110/sqrt3 | W011/sqrt3 | W101/sqrt3] bf16
    wc = nc.dram_tensor("wc", [MUL, mul4], dt.bfloat16, kind="ExternalInput")
    out = nc.dram_tensor("out", [npad, mul4], dt.float32, kind="ExternalOutput")

    with TileContext(nc) as tc:
        with (
            tc.tile_pool(name="consts", bufs=1) as consts,
            tc.tile_pool(name="xin", bufs=6) as xin,
            tc.tile_pool(name="outp", bufs=4) as outp,
            tc.tile_pool(name="ps", bufs=6, space="PSUM") as psp,
        ):
            w = consts.tile([MUL, mul4], dt.bfloat16)
            nc.sync.dma_start(out=w, in_=wc[:, :])

            for t in range(tiles):
                n0 = t * nt
                xytile = xin.tile([MUL, 8, nt], dt.bfloat16, tag="xy")
                nc.sync.dma_start(out=xytile, in_=xy[t])
                s_a0 = xytile[:, 0, :]
                s_a1 = xytile[:, 1, :]
                s_b1 = [xytile[:, 2 + k, :] for k in range(3)]
                s_c1 = [xytile[:, 5 + k, :] for k in range(3)]

                otile = outp.tile([MUL, slices, mul4], dt.float32, tag="ot")
                for si in range(slices):
                    ssl = slice(si * MUL, (si + 1) * MUL)
                    ps = psp.tile([MUL, mul4], dt.float32, tag="ps")
                    # o0 column range: a0@W000 + a1@W110s
                    nc.tensor.matmul(ps[:, 0:MUL], lhsT=s_a0[:, ssl],
                                     rhs=w[:, 0:MUL], start=True, stop=False)
                    nc.tensor.matmul(ps[:, 0:MUL], lhsT=s_a1[:, ssl],
                                     rhs=w[:, MUL:2 * MUL], start=False, stop=False)
                    # o1k blocked psum ranges: b1k@W011s + c1k@W101s
                    for k in range(3):
                        nc.tensor.matmul(ps[:, (1 + k) * MUL:(2 + k) * MUL],
                                         lhsT=s_b1[k][:, ssl],
                                         rhs=w[:, 2 * MUL:3 * MUL], start=False,
                                         stop=False)
                    for k in range(3):
                        nc.tensor.matmul(ps[:, (1 + k) * MUL:(2 + k) * MUL],
                                         lhsT=s_c1[k][:, ssl],
                                         rhs=w[:, 3 * MUL:mul4], start=False,
                                         stop=(k == 2))
                    # PSUM -> SBUF, interleaving o1 from k-blocked [k, w] to
                    # the reference's w-major layout (out col 128+3w+k).
                    # Alternate engines across slices to split the load.
                    eng = nc.scalar if si % 2 == 0 else nc.vector
                    ot = otile[:, si, :]
                    ps_v = ps[:, MUL:].rearrange("p (k w) -> p k w", k=3)
                    ot_v = ot[:, MUL:].rearrange("p (w k) -> p k w", k=3)
                    if si % 2 == 0:
                        nc.scalar.copy(out=ot[:, 0:MUL], in_=ps[:, 0:MUL])
                        nc.scalar.copy(out=ot_v, in_=ps_v)
                    else:
                        nc.vector.tensor_copy(ot[:, 0:MUL], ps[:, 0:MUL])
                        nc.vector.tensor_copy(ot_v, ps_v)
                # one ~1MB store for the whole tile: DRAM rows si*128+p
                nc.sync.dma_start(
                    out=out[n0:n0 + nt].rearrange("(si p) w -> p si w", p=MUL),
                    in_=otile)
    nc.compile()
    return nc


def _get_built():
    global _BUILT
    if _BUILT is None:
        _BUILT = _build()
    return _BUILT


def _prep_core_inputs(x, y, wc_bf, c):
    """Host-side shard prep: scale, transpose, deinterleave into bf16 blocks.

    Builds xy[tile, p, block, node] with the 8 pre-scaled activation blocks
    so each tile's inputs are one fully-contiguous ~1MB DMA.
    """
    bf16 = ml_dtypes.bfloat16
    lo, hi = c * PER, (c + 1) * PER
    xs = x[lo:hi]                          # [PER, 512] f32
    ys = y[lo:hi]                          # [PER, 4]   f32
    x0 = xs[:, :MUL]                       # [PER, 128]
    x1 = xs[:, MUL:].reshape(PER, MUL, 3)  # [PER, 128, 3]
    y0 = ys[:, 0:1]
    blocks = np.zeros((8, MUL, NPAD), dtype=bf16)
    blocks[0, :, :PER] = (y0 * x0).T
    blocks[1, :, :PER] = np.einsum('nuk,nk->un', x1, ys[:, 1:], optimize=True)
    for k in range(3):
        blocks[2 + k, :, :PER] = (ys[:, 1 + k:2 + k] * x0).T
        blocks[5 + k, :, :PER] = (y0 * x1[:, :, k]).T
    # [8, 128, tiles, NT] -> [tiles, 128, 8, NT]
    xy = np.ascontiguousarray(
        blocks.reshape(8, MUL, TILES, NT).transpose(2, 1, 0, 3))
    return {"xy": xy, "wc": wc_bf}


def kernel(x, y, W000, W011, W101, W110, b):
    from concourse.bass_utils import run_bass_kernel_spmd

    x = np.asarray(x, dtype=np.float32)
    y = np.asarray(y, dtype=np.float32)
    b = np.asarray(b, dtype=np.float32)
    inv_s3 = np.float32(1.0 / np.sqrt(3.0))
    w000 = np.asarray(W000, np.float32).reshape(MUL, MUL)
    w110 = inv_s3 * np.asarray(W110, np.float32).reshape(MUL, MUL)
    w011 = inv_s3 * np.asarray(W011, np.float32).reshape(MUL, MUL)
    w101 = inv_s3 * np.asarray(W101, np.float32).reshape(MUL, MUL)
    wc = np.concatenate([w000, w110, w011, w101], axis=1).astype(ml_dtypes.bfloat16)
    wc = np.ascontiguousarray(wc)

    nc = _get_built()
    in_maps = [_prep_core_inputs(x, y, wc, c) for c in range(NCORES)]
    res = run_bass_kernel_spmd(nc, in_maps, core_ids=list(range(NCORES)))
    global LAST_RESULT
    LAST_RESULT = res
    out = np.concatenate([r["out"][:PER] for r in res.results], axis=0)
    if b.any():
        out[:, :MUL] += b[None, :]
    return np.ascontiguousarray(out.astype(np.float32))
